# revision 30
# baseline (speedup 1.0000x reference)
"""Low-rank attention Trainium2 kernel (8 NeuronCores, SPMD).

Math (reference):
    tmp = relu(x @ W.T + b); U,V,Z,T = split(tmp, 4, axis=1)
    norm = sum(U @ colsum(V)) / n + eps ;  D = 1/norm
    out = concat[(U @ (V.T @ Z)) * D, T]

Sharding: rows of x across 8 cores. Per-core partials (V.T@Z [k,k],
colsum(V), colsum(U)) are AllReduced on-device; each core then computes
its local U @ (VtZ) * D.

Design notes (measured on trn2, ~338us vs 409us fp32r baseline):
- bf16 matmul operands, fp32 PSUM accumulation (~1.8e-3 rel err vs the
  2e-2 gate; PE streams ~2 rows/ns under the board power throttle).
- x^T fully resident in SBUF (16 KB/partition per d-tile x 8): the whole
  T-pass defers behind the AllReduce with zero HBM reloads.
- Split AllReduce: VtZ partial for ibs [0,12) reduces mid-phase-1 (absorbs
  the ~15-25us inter-core rendezvous while the DMA fabric is idle); the
  remainder + csu reduce at phase-1 end, hidden by the deferred T-pass.
- colsum(V) rides as a ones-column appended to Z inside the V^T@Z matmul.
- The deferred T matmuls read a gated copy of the T-weight columns (gate
  derived from csu), a true data dependency that stops the Tile scheduler
  from hoisting them out of the AllReduce window.
- Output DMAs batched 2 row-tiles per descriptor; PSUM->SBUF drains split
  across DVE and ACT.
- Phase 4 issues h-major over groups of 4 PSUM tiles (moving operand fixed
  across the group, start/stop pairs spread apart): 2-mm groups with
  alternating moving operands measured at HALF the PE issue rate.
"""
import sys

sys.path.insert(0, "/opt/trn_rl_repo")
import numpy as np
import ml_dtypes

BF16 = ml_dtypes.bfloat16

NCORES = 8
N_ROWS, D_IN, K = 65536, 1024, 256
NLOC = N_ROWS // NCORES      # 8192 rows per core
P = 128
IB = 512                     # i-block width
NB = NLOC // IB              # 16 blocks
EPS = 1e-6
TDEF = 16                    # T-pass blocks deferred to overlap the AllReduce
XCHUNKS = [(0, 512), (512, 512), (1024, 1024), (2048, 2048), (4096, 2048), (6144, 2048)]

_built = {}


def _build(d_rows):
    import concourse.bacc as bacc
    import concourse.mybir as mybir
    import concourse.tile as tile

    dt = mybir.dt
    f32, bf16 = dt.float32, dt.bfloat16
    RELU = mybir.ActivationFunctionType.Relu
    DT = d_rows // P
    NSUB = IB // P

    nc = bacc.Bacc("TRN2", target_bir_lowering=False, debug=False, num_devices=NCORES)
    xT = nc.dram_tensor("xT", [d_rows, NLOC], bf16, kind="ExternalInput")
    WT = nc.dram_tensor("WT", [d_rows, 4 * K], bf16, kind="ExternalInput")
    out = nc.dram_tensor("out", [NLOC, 2 * K], f32, kind="ExternalOutput")

    with tile.TileContext(nc) as tc:
        with (
            tc.tile_pool(name="wp", bufs=1) as wp,
            tc.tile_pool(name="xp", bufs=1) as xp,
            tc.tile_pool(name="up", bufs=1) as up,
            tc.tile_pool(name="vzp", bufs=5) as vzp,
            tc.tile_pool(name="ob", bufs=6) as ob,
            tc.tile_pool(name="acc", bufs=1) as accp,
            tc.tile_pool(name="ps", bufs=8, space="PSUM") as ps,
            tc.tile_pool(name="dram", bufs=1, space="DRAM") as dram,
        ):
            # Weights (split gpsimd/scalar queues) and resident x^T (sync queue,
            # kd-interleaved chunks, small first chunks so ib0 can start early).
            # W loads split column-wise: U-cols first (ib0's first matmuls),
            # V|Z next; the T-cols are not needed until the deferred T-pass
            # (phase-1 end), so they leave the critical priming bandwidth and
            # issue after the early x chunks.
            wt = []
            for kd in range(DT):
                w = wp.tile([P, 4 * K], bf16, tag=f"w{kd}", name=f"w{kd}")
                nc.sync.dma_start(
                    out=w[:, 0:K], in_=WT[kd * P:(kd + 1) * P, 0:K])
                q = nc.gpsimd if kd < DT // 2 else nc.scalar
                q.dma_start(
                    out=w[:, K:3 * K], in_=WT[kd * P:(kd + 1) * P, K:3 * K])
                wt.append(w)
            xf = [xp.tile([P, NLOC], bf16, tag=f"x{kd}", name=f"x{kd}") for kd in range(DT)]
            for ci, (c0, cw) in enumerate(XCHUNKS):
                for kd in range(DT):
                    nc.sync.dma_start(
                        out=xf[kd][:, c0:c0 + cw],
                        in_=xT[kd * P:(kd + 1) * P, c0:c0 + cw],
                    )
                if ci == 2:
                    for kd in range(DT):
                        nc.sync.dma_start(
                            out=wt[kd][:, 3 * K:4 * K],
                            in_=WT[kd * P:(kd + 1) * P, 3 * K:4 * K])
            ones_row = wp.tile([1, P], f32, tag="ones_row")
            nc.vector.memset(ones_row[:], 1.0)

            ut = [up.tile([P, NLOC], bf16, tag=f"ut{h}", name=f"ut{h}") for h in range(2)]
            csu_cols = [accp.tile([P, NB], f32, tag=f"csuc{h}", name=f"csuc{h}") for h in range(2)]
            # vtz_acc column 256 carries the colsum(V) partial (ones-column
            # trick). Two accumulators: A covers ibs [0, SPLIT), AllReduced
            # mid-phase-1 (absorbing the inter-core rendezvous while the PE is
            # still busy and the DMA fabric idle); B covers the rest + csu.
            SPLIT = 12
            vtz_acc = [
                [accp.tile([P, K + 1], f32, tag=f"vtz{ab}{h}", name=f"vtz{ab}{h}")
                 for h in range(2)] for ab in range(2)
            ]
            bin_a = dram.tile([2 * P, K + 1], f32)
            bout_a = dram.tile([2 * P, K + 1], f32)

            def t_pass(ib, wsrc):
                """T = relu(x @ Wt): 4 row-subtiles, one batched out-DMA."""
                for g in range(NSUB // 2):
                    otb = ob.tile([P, 2, K], f32, tag="ob")
                    for s2 in range(2):
                        s = g * 2 + s2
                        pt = ps.tile([P, K], f32, tag="work")
                        for kd in range(DT):
                            nc.tensor.matmul(
                                pt[:], xf[kd][:, ib * IB + s * P:ib * IB + (s + 1) * P],
                                wsrc[kd],
                                start=(kd == 0), stop=(kd == DT - 1),
                            )
                        nc.vector.tensor_relu(otb[:, s2, :], pt[:])
                    i0 = ib * IB + g * 2 * P
                    nc.sync.dma_start(
                        out=out[i0:i0 + 2 * P, K:2 * K].rearrange(
                            "(s p) c -> p s c", p=P),
                        in_=otb[:],
                    )

            wt_t = [wt[kd][:, 3 * K:4 * K] for kd in range(DT)]

            # ---- phase 1: projection + partial reductions ----
            for ib in range(NB):
                # U^T [k1, i] — stationary Wu^T, moving x^T; relu on ACT with
                # free-dim running sum (colsum_U partial) via accum_out.
                for h in range(2):
                    pu = ps.tile([P, IB], f32, tag="work")
                    for kd in range(DT):
                        nc.tensor.matmul(
                            pu[:], wt[kd][:, h * P:(h + 1) * P],
                            xf[kd][:, ib * IB:(ib + 1) * IB],
                            start=(kd == 0), stop=(kd == DT - 1),
                        )
                    nc.scalar.activation(
                        ut[h][:, ib * IB:(ib + 1) * IB], pu[:], RELU,
                        accum_out=csu_cols[h][:, ib:ib + 1],
                    )
                # V|Z in natural [i, j] layout per 128-row subtile; col 512 = 1.0
                vz_tiles = []
                for s in range(NSUB):
                    pvz = ps.tile([P, IB], f32, tag="work")
                    for kd in range(DT):
                        nc.tensor.matmul(
                            pvz[:], xf[kd][:, ib * IB + s * P:ib * IB + (s + 1) * P],
                            wt[kd][:, K:3 * K],
                            start=(kd == 0), stop=(kd == DT - 1),
                        )
                    vz = vzp.tile([P, 2 * K + 2], bf16, tag="vz")
                    nc.vector.tensor_relu(vz[:, 0:2 * K], pvz[:])
                    nc.vector.memset(vz[:, 2 * K:2 * K + 1], 1.0)
                    vz_tiles.append(vz)
                if ib < NB - TDEF:
                    t_pass(ib, wt_t)
                # VtZ|csV partial: contract i (partitions) over this block
                ab = 0 if ib < SPLIT else 1
                for h in range(2):
                    pz = ps.tile([P, K + 1], f32, tag="work")
                    for s in range(NSUB):
                        nc.tensor.matmul(
                            pz[:], vz_tiles[s][:, h * P:(h + 1) * P],
                            vz_tiles[s][:, K:2 * K + 1],
                            start=(s == 0), stop=(s == NSUB - 1),
                        )
                    if ib in (0, SPLIT):
                        nc.vector.tensor_copy(vtz_acc[ab][h][:], pz[:])
                    else:
                        nc.vector.tensor_add(vtz_acc[ab][h][:], vtz_acc[ab][h][:], pz[:])
                if ib == SPLIT - 1:
                    # launch AllReduce A: covers the bulk of VtZ while ibs
                    # [SPLIT, NB) still compute
                    for h in range(2):
                        nc.scalar.dma_start(
                            out=bin_a[h * P:(h + 1) * P, :], in_=vtz_acc[0][h][:]
                        )
                    nc.gpsimd.collective_compute(
                        "AllReduce", mybir.AluOpType.add,
                        replica_groups=[list(range(NCORES))],
                        ins=[bin_a.opt()], outs=[bout_a.opt()],
                    )

            # ---- phase 2: AllReduce [2*[k,k+1]] + [2*[k]] partials ----
            # Staging DMAs ride the scalar queue so they never wait behind the
            # sync queue's bulk traffic.
            csu = [accp.tile([P, 1], f32, tag=f"csu{h}", name=f"csu{h}") for h in range(2)]
            for h in range(2):
                nc.vector.reduce_sum(csu[h][:], csu_cols[h][:], axis=mybir.AxisListType.X)
            # Copy of the T-weight columns gated on a phase-1 output (gate==1.0
            # exactly, computed from csu): the deferred T matmuls read these
            # tiles, which truly pins them after phase 1 so they land inside
            # the AllReduce window instead of being hoisted into phase 1.
            gate = accp.tile([P, 1], f32, tag="gate")
            nc.vector.tensor_scalar(
                out=gate[:], in0=csu[0][:], scalar1=0.0, scalar2=1.0,
                op0=mybir.AluOpType.mult, op1=mybir.AluOpType.add,
            )
            wt2 = [wp.tile([P, K], bf16, tag=f"w2_{kd}", name=f"w2_{kd}") for kd in range(DT)]
            for kd in range(DT):
                nc.vector.tensor_scalar_mul(wt2[kd][:], wt[kd][:, 3 * K:4 * K], gate[:])
            bin_ = dram.tile([2 * P + 2, K + 1], f32)
            bout = dram.tile([2 * P + 2, K + 1], f32)
            for h in range(2):
                nc.scalar.dma_start(out=bin_[h * P:(h + 1) * P, :], in_=vtz_acc[1][h][:])
            for h in range(2):
                nc.scalar.dma_start(
                    out=bin_[2 * P + h, 0:P].rearrange("(p one) -> p one", one=1),
                    in_=csu[h][:],
                )
            nc.gpsimd.collective_compute(
                "AllReduce", mybir.AluOpType.add,
                replica_groups=[list(range(NCORES))],
                ins=[bin_.opt()], outs=[bout.opt()],
            )
            # ---- deferred T-pass: keeps PE busy/warm during the AllReduce ----
            wt2_t = [wt2[kd][:] for kd in range(DT)]
            for ib in range(NB - TDEF, NB):
                t_pass(ib, wt2_t)

            # ---- phase 3: D = 1/(csU.csV/n + eps); scale VtZ ----
            vtzf = [accp.tile([P, K + 1], f32, tag=f"vtzf{h}", name=f"vtzf{h}") for h in range(2)]
            vtzfb = accp.tile([P, K + 1], f32, tag="vtzfb")
            for h in range(2):
                nc.scalar.dma_start(out=vtzf[h][:], in_=bout_a[h * P:(h + 1) * P, :])
            for h in range(2):
                nc.scalar.dma_start(out=vtzfb[:], in_=bout[h * P:(h + 1) * P, :])
                nc.vector.tensor_add(vtzf[h][:], vtzf[h][:], vtzfb[:])
            csut = accp.tile([P, 2], f32, tag="csut")
            nc.scalar.dma_start(
                out=csut[:], in_=bout[2 * P:2 * P + 2, 0:P].rearrange("t p -> p t")
            )
            csvt = accp.tile([P, 2], f32, tag="csvt")
            for h in range(2):
                nc.vector.tensor_copy(csvt[:, h:h + 1], vtzf[h][:, K:K + 1])
            pdot = ps.tile([1, 1], f32, tag="work")
            for h in range(2):
                nc.tensor.matmul(
                    pdot[:], csut[:, h:h + 1], csvt[:, h:h + 1],
                    start=(h == 0), stop=(h == 1),
                )
            dsb = accp.tile([1, 1], f32, tag="dsb")
            nc.vector.tensor_scalar(
                out=dsb[:], in0=pdot[:], scalar1=1.0 / N_ROWS, scalar2=EPS,
                op0=mybir.AluOpType.mult, op1=mybir.AluOpType.add,
            )
            nc.vector.reciprocal(dsb[:], dsb[:])
            pb = ps.tile([P, 1], f32, tag="work")
            nc.tensor.matmul(pb[:], ones_row[:], dsb[:], start=True, stop=True)
            dbc = accp.tile([P, 1], f32, tag="dbc")
            nc.vector.tensor_copy(dbc[:], pb[:])
            vtzr = [accp.tile([P, K], bf16, tag=f"vtzr{h}", name=f"vtzr{h}") for h in range(2)]
            for h in range(2):
                nc.vector.tensor_scalar_mul(vtzr[h][:], vtzf[h][:, 0:K], dbc[:])

            # ---- phase 4: res = U @ (VtZ * D), batched row-natural writes ----
            # h-major over groups of 8 row-tiles: the moving operand stays
            # fixed for 8 consecutive matmuls and each PSUM start/stop pair is
            # spread apart, keeping the weight path warm.
            GG = 4
            for gb in range(NLOC // P // GG):
                prs = [ps.tile([P, K], f32, tag="work", name=f"pr{t}") for t in range(GG)]
                for h in range(2):
                    for t in range(GG):
                        i0 = (gb * GG + t) * P
                        nc.tensor.matmul(
                            prs[t][:], ut[h][:, i0:i0 + P], vtzr[h][:],
                            start=(h == 0), stop=(h == 1),
                        )
                for g2 in range(GG // 2):
                    orb = ob.tile([P, 2, K], f32, tag="ob")
                    for s2 in range(2):
                        t = g2 * 2 + s2
                        # split PSUM->SBUF copies across DVE and ACT: either
                        # alone is slower than the PE here
                        if s2 == 0:
                            nc.vector.tensor_copy(orb[:, s2, :], prs[t][:])
                        else:
                            nc.scalar.copy(orb[:, s2, :], prs[t][:])
                    i0 = (gb * GG + g2 * 2) * P
                    nc.sync.dma_start(
                        out=out[i0:i0 + 2 * P, 0:K].rearrange(
                            "(s p) c -> p s c", p=P),
                        in_=orb[:],
                    )

    nc.compile()
    return nc


def _get_nc(d_rows):
    if d_rows not in _built:
        _built[d_rows] = _build(d_rows)
    return _built[d_rows]


def _run(x, W, b, trace=False, trace_cores=None):
    from concourse.bass_utils import run_bass_kernel_spmd

    x = np.ascontiguousarray(x, dtype=np.float32)
    W = np.ascontiguousarray(W, dtype=np.float32)
    b = np.asarray(b, dtype=np.float32)
    if np.any(b):
        d_rows = 1152  # pad contraction: extra ones-row in x picks up b from W
        WT_full = np.zeros((d_rows, 4 * K), np.float32)
        WT_full[:D_IN] = W.T
        WT_full[D_IN] = b
    else:
        d_rows = D_IN
        WT_full = np.ascontiguousarray(W.T)
    WT_bf = WT_full.astype(BF16)
    nc = _get_nc(d_rows)
    in_maps = []
    for c in range(NCORES):
        xs = x[c * NLOC:(c + 1) * NLOC]
        if d_rows == D_IN:
            xTs = np.ascontiguousarray(xs.T.astype(BF16))
        else:
            xTs = np.zeros((d_rows, NLOC), BF16)
            xTs[:D_IN] = xs.T.astype(BF16)
            xTs[D_IN] = 1.0
        in_maps.append({"xT": xTs, "WT": WT_bf})
    res = run_bass_kernel_spmd(
        nc, in_maps, list(range(NCORES)),
        trace=trace, **({"trace_cores": trace_cores} if trace_cores else {}),
    )
    full = np.concatenate([res.results[c]["out"] for c in range(NCORES)], axis=0)
    return full, res


def kernel(x, W, b):
    full, _ = _run(x, W, b)
    return full
